# revision 1
# baseline (speedup 1.0000x reference)
"""Trainium2 Bass kernel for nn_LocalEnhancementModule (8-core SPMD, data-parallel over batch).

Per-sample computation (B=16, P=256 patches, D=4096, E=512):
    p      = patchify(x)                       [P, D]
    theta  = p @ theta_w + theta_b             [P, E]
    f      = p @ f_w + f_b                     [P, E]
    wgt    = softmax(theta @ f.T, axis=-1)     [P, P]
    g      = p @ g_w + g_b                     [P, D]
    out    = unpatchify(wgt[:,None,:] * g.reshape(P,C,P)) * scale + x

Sharding: 2 samples per core. Host pre-patchifies x and uploads three dense
layouts (p_nat fp32 for the residual, pT as float32r for the theta/f/score
matmuls, pT as bf16 for the g matmul). scale is folded into g_w on the host.
Matmul dtypes: float32r (tf32-class, 1 cyc/row at N>=256) on the attention
path, bf16 on the 4096x4096 g projection.
"""

import sys
import numpy as np

try:
    import concourse.bacc as bacc
except ImportError:  # pragma: no cover
    for _p in ("/opt/trn_rl_repo", "/root/.axon_site/_ro/trn_rl_repo"):
        if _p not in sys.path:
            sys.path.append(_p)
    import concourse.bacc as bacc
import concourse.mybir as mybir
import concourse.tile as tile
from concourse.bass_utils import run_bass_kernel_spmd
import ml_dtypes

NCORES = 8
B, C, H, W = 16, 16, 256, 256
NPS, PH, PW = 16, 16, 16
P = NPS * NPS            # 256 patches
D = C * PH * PW          # 4096
E = 512
SPC = B // NCORES        # 2 samples per core
PP = SPC * P             # 512 patch rows per core
KT = D // 128            # 32 contraction tiles
ET = E // 128            # 4 embedding chunks
DCH = D // 512           # 8 output-column chunks for g
GRP = [(s, pc) for s in range(SPC) for pc in range(2)]

F32 = mybir.dt.float32
F32R = mybir.dt.float32r
BF16 = mybir.dt.bfloat16

_built = {}
LAST_RESULTS = None  # stashed BassKernelResults for test harness introspection


def _build(with_tb, with_fb, with_gb):
    key = (with_tb, with_fb, with_gb)
    if key in _built:
        return _built[key]

    nc = bacc.Bacc("TRN2", num_devices=NCORES, debug=False)
    pt32_d = nc.dram_tensor("pt32", [D, PP], F32R, kind="ExternalInput").ap()
    ptbf_d = nc.dram_tensor("ptbf", [D, PP], BF16, kind="ExternalInput").ap()
    pnat_d = nc.dram_tensor("pnat", [PP, D], F32, kind="ExternalInput").ap()
    tw_d = nc.dram_tensor("tw", [D, E], F32R, kind="ExternalInput").ap()
    fw_d = nc.dram_tensor("fw", [D, E], F32R, kind="ExternalInput").ap()
    gw_d = nc.dram_tensor("gw", [D, D], BF16, kind="ExternalInput").ap()
    tb_d = nc.dram_tensor("tb", [E, 1], F32, kind="ExternalInput").ap() if with_tb else None
    fb_d = nc.dram_tensor("fb", [E, 1], F32, kind="ExternalInput").ap() if with_fb else None
    gb_d = nc.dram_tensor("gb", [1, D], F32, kind="ExternalInput").ap() if with_gb else None
    out_d = nc.dram_tensor("out", [PP, D], F32, kind="ExternalOutput").ap()

    with tile.TileContext(nc) as tc:
        with tc.tile_pool(name="persist", bufs=1) as pp_, \
             tc.tile_pool(name="wstream", bufs=16) as wp, \
             tc.tile_pool(name="gstream", bufs=8) as gp, \
             tc.tile_pool(name="pnstream", bufs=4) as pnp, \
             tc.tile_pool(name="enh", bufs=6) as ep, \
             tc.tile_pool(name="sm", bufs=2) as smp, \
             tc.tile_pool(name="psum", bufs=1, space="PSUM") as psp:

            # resident pT tiles (k-major)
            pt32 = []
            ptbf = []
            for k in range(KT):
                t32 = pp_.tile([128, PP], F32R, name=f"pt32_{k}", tag=f"pt32_{k}")
                nc.sync.dma_start(out=t32[:, :], in_=pt32_d[k * 128:(k + 1) * 128, :])
                pt32.append(t32)
            for k in range(KT):
                tbf = pp_.tile([128, PP], BF16, name=f"ptbf_{k}", tag=f"ptbf_{k}")
                nc.sync.dma_start(out=tbf[:, :], in_=ptbf_d[k * 128:(k + 1) * 128, :])
                ptbf.append(tbf)

            bias_sb = {0: [], 1: []}
            for wi, bd in ((0, tb_d), (1, fb_d)):
                if bd is None:
                    continue
                for e in range(ET):
                    bt = pp_.tile([128, 1], F32, name=f"bias_{wi}_{e}", tag=f"bias_{wi}_{e}")
                    nc.sync.dma_start(out=bt[:, :], in_=bd[e * 128:(e + 1) * 128, :])
                    bias_sb[wi].append(bt)
            gb_sb = None
            if gb_d is not None:
                gb_sb = pp_.tile([1, D], F32, name="gb_sb", tag="gb_sb")
                nc.sync.dma_start(out=gb_sb[:, :], in_=gb_d[:, :])

            # ---- theta / f projections: out projT[(w,e)] = [128(e), PP] ----
            proj_sb = {}
            for wi, wd in ((0, tw_d), (1, fw_d)):
                for e in range(ET):
                    ps = psp.tile([128, PP], F32, name=f"ps_attn_{wi}_{e}", tag="attn", bufs=2)
                    for k in range(KT):
                        wt = wp.tile([128, 128], F32R, name=f"wt_{wi}_{e}_{k}", tag="w")
                        nc.sync.dma_start(
                            out=wt[:, :],
                            in_=wd[k * 128:(k + 1) * 128, e * 128:(e + 1) * 128])
                        nc.tensor.matmul(ps[:, :], wt[:, :], pt32[k][:, :],
                                         start=(k == 0), stop=(k == KT - 1))
                    sb = pp_.tile([128, PP], F32R, name=f"proj_{wi}_{e}", tag=f"proj_{wi}_{e}")
                    if bias_sb[wi]:
                        nc.vector.tensor_scalar_add(sb[:, :], ps[:, :], bias_sb[wi][e][:, :])
                    else:
                        nc.vector.tensor_copy(sb[:, :], ps[:, :])
                    proj_sb[(wi, e)] = sb

            # ---- scores + softmax per (sample, p-chunk) ----
            wgt = {}
            for (s, pc) in GRP:
                sps = psp.tile([128, P], F32, name=f"ps_sc_{s}_{pc}", tag="sc", bufs=2)
                col = s * P + pc * 128
                for e in range(ET):
                    nc.tensor.matmul(sps[:, :],
                                     proj_sb[(0, e)][:, col:col + 128],
                                     proj_sb[(1, e)][:, s * P:(s + 1) * P],
                                     start=(e == 0), stop=(e == ET - 1))
                mx = smp.tile([128, 1], F32, name=f"mx_{s}_{pc}", tag="mx")
                nc.vector.tensor_reduce(out=mx[:, :], in_=sps[:, :],
                                        axis=mybir.AxisListType.X, op=mybir.AluOpType.max)
                ngm = smp.tile([128, 1], F32, name=f"ngm_{s}_{pc}", tag="ngm")
                nc.vector.tensor_scalar_mul(ngm[:, :], mx[:, :], -1.0)
                ex = smp.tile([128, P], F32, name=f"ex_{s}_{pc}", tag="ex")
                ssum = smp.tile([128, 1], F32, name=f"ssum_{s}_{pc}", tag="ssum")
                nc.scalar.activation(ex[:, :], sps[:, :], mybir.ActivationFunctionType.Exp,
                                     bias=ngm[:, :], scale=1.0, accum_out=ssum[:, :])
                rec = smp.tile([128, 1], F32, name=f"rec_{s}_{pc}", tag="rec")
                nc.vector.reciprocal(rec[:, :], ssum[:, :])
                wt_ = pp_.tile([128, P], F32, name=f"wgt_{s}_{pc}", tag=f"wgt_{s}_{pc}")
                nc.vector.tensor_scalar_mul(wt_[:, :], ex[:, :], rec[:, :])
                wgt[(s, pc)] = wt_

            # ---- g projection + gating + residual ----
            for d in range(DCH):
                dcol = d * 512
                gps = {}
                for (s, pc) in GRP:
                    gps[(s, pc)] = psp.tile([128, 512], F32,
                                            name=f"ps_g_{d}_{s}_{pc}", tag="g", bufs=4)
                for k in range(KT):
                    gt = gp.tile([128, 512], BF16, name=f"gt_{d}_{k}", tag="gt")
                    nc.sync.dma_start(out=gt[:, :],
                                      in_=gw_d[k * 128:(k + 1) * 128, dcol:dcol + 512])
                    for (s, pc) in GRP:
                        col = s * P + pc * 128
                        nc.tensor.matmul(gps[(s, pc)][:, :], ptbf[k][:, col:col + 128],
                                         gt[:, :], start=(k == 0), stop=(k == KT - 1))
                for (s, pc) in GRP:
                    row = s * P + pc * 128
                    g_ps = gps[(s, pc)]
                    if gb_sb is not None:
                        nc.vector.tensor_add(g_ps[:, :], g_ps[:, :],
                                             gb_sb[0:1, dcol:dcol + 512].partition_broadcast(128))
                    en = ep.tile([128, 512], F32, name=f"en_{d}_{s}_{pc}", tag="en")
                    nc.vector.tensor_mul(en[:, 0:256], g_ps[:, 0:256], wgt[(s, pc)][:, :])
                    nc.vector.tensor_mul(en[:, 256:512], g_ps[:, 256:512], wgt[(s, pc)][:, :])
                    pn = pnp.tile([128, 512], F32, name=f"pn_{d}_{s}_{pc}", tag="pn")
                    nc.sync.dma_start(out=pn[:, :], in_=pnat_d[row:row + 128, dcol:dcol + 512])
                    nc.vector.tensor_add(en[:, :], en[:, :], pn[:, :])
                    nc.sync.dma_start(out=out_d[row:row + 128, dcol:dcol + 512], in_=en[:, :])

    nc.compile()
    _built[key] = nc
    return nc


def kernel(**inputs):
    global LAST_RESULTS
    x = np.ascontiguousarray(inputs["x"], dtype=np.float32)
    tw = np.ascontiguousarray(inputs["theta_w"], dtype=np.float32)
    fw = np.ascontiguousarray(inputs["f_w"], dtype=np.float32)
    gw = np.ascontiguousarray(inputs["g_w"], dtype=np.float32)
    tb = np.asarray(inputs["theta_b"], dtype=np.float32)
    fb = np.asarray(inputs["f_b"], dtype=np.float32)
    gb = np.asarray(inputs["g_b"], dtype=np.float32)
    scale = float(np.asarray(inputs["scale"], dtype=np.float32).reshape(-1)[0])

    with_tb = bool(np.any(tb))
    with_fb = bool(np.any(fb))
    with_gb = bool(np.any(gb))
    nc = _build(with_tb, with_fb, with_gb)

    # patchify: [B,C,H,W] -> [B,P,D] with D ordered (c, u, v)
    p = x.reshape(B, C, NPS, PH, NPS, PW).transpose(0, 2, 4, 1, 3, 5).reshape(B, P, D)
    gw_b = np.ascontiguousarray((scale * gw)).astype(ml_dtypes.bfloat16)
    in_maps = []
    for ci in range(NCORES):
        p2 = p[ci * SPC:(ci + 1) * SPC]                      # [SPC, P, D]
        pnat = np.ascontiguousarray(p2.reshape(PP, D), dtype=np.float32)
        pT = np.ascontiguousarray(p2.transpose(2, 0, 1).reshape(D, PP), dtype=np.float32)
        m = {"pt32": pT, "ptbf": pT.astype(ml_dtypes.bfloat16), "pnat": pnat,
             "tw": tw, "fw": fw, "gw": gw_b}
        if with_tb:
            m["tb"] = np.ascontiguousarray(tb.reshape(E, 1))
        if with_fb:
            m["fb"] = np.ascontiguousarray(fb.reshape(E, 1))
        if with_gb:
            m["gb"] = np.ascontiguousarray((scale * gb).reshape(1, D))
        in_maps.append(m)

    res = run_bass_kernel_spmd(nc, in_maps, core_ids=list(range(NCORES)))
    LAST_RESULTS = res
    o = np.concatenate([res.results[ci]["out"].reshape(SPC, P, D)
                        for ci in range(NCORES)], axis=0)     # [B, P, D]
    img = (o.reshape(B, NPS, NPS, C, PH, PW)
            .transpose(0, 3, 1, 4, 2, 5)
            .reshape(B, C, H, W))
    return np.ascontiguousarray(img, dtype=np.float32)


# revision 2
# speedup vs baseline: 1.2818x; 1.2818x over previous
"""Trainium2 Bass kernel for nn_LocalEnhancementModule (8-core SPMD, data-parallel over batch).

Per-sample computation (B=16, P=256 patches, D=4096, E=512):
    p      = patchify(x)                       [P, D]
    theta  = p @ theta_w + theta_b             [P, E]
    f      = p @ f_w + f_b                     [P, E]
    wgt    = softmax(theta @ f.T, axis=-1)     [P, P]
    g      = p @ g_w + g_b                     [P, D]
    out    = unpatchify(wgt[:,None,:] * g.reshape(P,C,P)) * scale + x

Sharding: 2 samples per core. Host pre-patchifies x and uploads three dense
layouts (p_nat fp32 for the residual, pT as float32r for the theta/f/score
matmuls, pT as bf16 for the g matmul). scale is folded into g_w on the host.
Matmul dtypes: float32r (tf32-class) on the attention path, bf16 on the
4096x4096 g projection.

Schedule: theta/f run k-outer into 8 PSUM banks (weights stream as full
[128,512] row-tiles); scores+softmax; then the g projection streams g_w
column-pair slabs with 8 PSUM accumulators per d-pair round. DMA issue is
split across both HWDGE rings (sync: pt32/theta_w/f_w/g_w; scalar: ptbf/
p_nat/out) so neither ring's serial issue rate paces the PE.
"""

import sys
import numpy as np

try:
    import concourse.bacc as bacc
except ImportError:  # pragma: no cover
    for _p in ("/opt/trn_rl_repo", "/root/.axon_site/_ro/trn_rl_repo"):
        if _p not in sys.path:
            sys.path.append(_p)
    import concourse.bacc as bacc
import concourse.mybir as mybir
import concourse.tile as tile
from concourse.bass_utils import run_bass_kernel_spmd
import ml_dtypes

NCORES = 8
B, C, H, W = 16, 16, 256, 256
NPS, PH, PW = 16, 16, 16
P = NPS * NPS            # 256 patches
D = C * PH * PW          # 4096
E = 512
SPC = B // NCORES        # 2 samples per core
PP = SPC * P             # 512 patch rows per core
KT = D // 128            # 32 contraction tiles
ET = E // 128            # 4 embedding chunks
DP = D // 1024           # 4 column-pair slabs for g
GRP = [(s, pc) for s in range(SPC) for pc in range(2)]

F32 = mybir.dt.float32
F32R = mybir.dt.float32r
BF16 = mybir.dt.bfloat16

_built = {}
LAST_RESULTS = None  # stashed BassKernelResults for test harness introspection


def _build(with_tb, with_fb, with_gb):
    key = (with_tb, with_fb, with_gb)
    if key in _built:
        return _built[key]

    nc = bacc.Bacc("TRN2", num_devices=NCORES, debug=False)
    pt32_d = nc.dram_tensor("pt32", [D, PP], F32R, kind="ExternalInput").ap()
    ptbf_d = nc.dram_tensor("ptbf", [D, PP], BF16, kind="ExternalInput").ap()
    pnat_d = nc.dram_tensor("pnat", [PP, D], F32, kind="ExternalInput").ap()
    tw_d = nc.dram_tensor("tw", [D, E], F32R, kind="ExternalInput").ap()
    fw_d = nc.dram_tensor("fw", [D, E], F32R, kind="ExternalInput").ap()
    gw_d = nc.dram_tensor("gw", [D, D], BF16, kind="ExternalInput").ap()
    tb_d = nc.dram_tensor("tb", [E, 1], F32, kind="ExternalInput").ap() if with_tb else None
    fb_d = nc.dram_tensor("fb", [E, 1], F32, kind="ExternalInput").ap() if with_fb else None
    gb_d = nc.dram_tensor("gb", [1, D], F32, kind="ExternalInput").ap() if with_gb else None
    out_d = nc.dram_tensor("out", [PP, D], F32, kind="ExternalOutput").ap()

    with tile.TileContext(nc) as tc:
        with tc.tile_pool(name="persist", bufs=1) as pp_, \
             tc.tile_pool(name="wstream", bufs=6) as wp, \
             tc.tile_pool(name="gstream", bufs=6) as gp, \
             tc.tile_pool(name="pnstream", bufs=3) as pnp, \
             tc.tile_pool(name="enh", bufs=3) as ep, \
             tc.tile_pool(name="sm", bufs=2) as smp:

            # ptbf prefetch on the scalar HWDGE ring (runs during theta/f)
            ptbf = []
            for k in range(KT):
                tbf = pp_.tile([128, PP], BF16, name=f"ptbf_{k}", tag=f"ptbf_{k}")
                nc.scalar.dma_start(out=tbf[:, :], in_=ptbf_d[k * 128:(k + 1) * 128, :])
                ptbf.append(tbf)

            bias_sb = {0: [], 1: []}
            for wi, bd in ((0, tb_d), (1, fb_d)):
                if bd is None:
                    continue
                for e in range(ET):
                    bt = pp_.tile([128, 1], F32, name=f"bias_{wi}_{e}", tag=f"bias_{wi}_{e}")
                    nc.scalar.dma_start(out=bt[:, :], in_=bd[e * 128:(e + 1) * 128, :])
                    bias_sb[wi].append(bt)
            gb_sb = None
            if gb_d is not None:
                gb_sb = pp_.tile([1, D], F32, name="gb_sb", tag="gb_sb")
                nc.scalar.dma_start(out=gb_sb[:, :], in_=gb_d[:, :])

            # ---- theta / f projections, k-outer into 8 PSUM banks ----
            # projT[(w,e)] = [128(e), PP]  (thetaT / fT, float32r)
            pt32 = []
            with tc.tile_pool(name="psA", bufs=1, space="PSUM") as psA:
                ps_attn = {}
                for wi in (0, 1):
                    for e in range(ET):
                        ps_attn[(wi, e)] = psA.tile([128, PP], F32,
                                                    name=f"ps_attn_{wi}_{e}",
                                                    tag=f"attn_{wi}_{e}")
                for k in range(KT):
                    t32 = pp_.tile([128, PP], F32R, name=f"pt32_{k}", tag=f"pt32_{k}")
                    nc.sync.dma_start(out=t32[:, :], in_=pt32_d[k * 128:(k + 1) * 128, :])
                    pt32.append(t32)
                    for wi, wd in ((0, tw_d), (1, fw_d)):
                        wt = wp.tile([128, E], F32R, name=f"wt_{wi}_{k}", tag="w")
                        nc.sync.dma_start(out=wt[:, :], in_=wd[k * 128:(k + 1) * 128, :])
                        for e in range(ET):
                            nc.tensor.matmul(ps_attn[(wi, e)][:, :],
                                             wt[:, e * 128:(e + 1) * 128],
                                             t32[:, :],
                                             start=(k == 0), stop=(k == KT - 1))
                proj_sb = {}
                for wi in (0, 1):
                    for e in range(ET):
                        sb = pp_.tile([128, PP], F32R, name=f"proj_{wi}_{e}",
                                      tag=f"proj_{wi}_{e}")
                        if bias_sb[wi]:
                            nc.vector.tensor_scalar_add(sb[:, :], ps_attn[(wi, e)][:, :],
                                                        bias_sb[wi][e][:, :])
                        else:
                            nc.vector.tensor_copy(sb[:, :], ps_attn[(wi, e)][:, :])
                        proj_sb[(wi, e)] = sb

            # ---- scores + softmax per (sample, p-chunk) ----
            wgt = {}
            with tc.tile_pool(name="psB", bufs=1, space="PSUM") as psB:
                for (s, pc) in GRP:
                    sps = psB.tile([128, P], F32, name=f"ps_sc_{s}_{pc}", tag="sc", bufs=2)
                    col = s * P + pc * 128
                    for e in range(ET):
                        nc.tensor.matmul(sps[:, :],
                                         proj_sb[(0, e)][:, col:col + 128],
                                         proj_sb[(1, e)][:, s * P:(s + 1) * P],
                                         start=(e == 0), stop=(e == ET - 1))
                    mx = smp.tile([128, 1], F32, name=f"mx_{s}_{pc}", tag="mx")
                    nc.vector.tensor_reduce(out=mx[:, :], in_=sps[:, :],
                                            axis=mybir.AxisListType.X, op=mybir.AluOpType.max)
                    ngm = smp.tile([128, 1], F32, name=f"ngm_{s}_{pc}", tag="ngm")
                    nc.vector.tensor_scalar_mul(ngm[:, :], mx[:, :], -1.0)
                    ex = smp.tile([128, P], F32, name=f"ex_{s}_{pc}", tag="ex")
                    ssum = smp.tile([128, 1], F32, name=f"ssum_{s}_{pc}", tag="ssum")
                    nc.scalar.activation(ex[:, :], sps[:, :], mybir.ActivationFunctionType.Exp,
                                         bias=ngm[:, :], scale=1.0, accum_out=ssum[:, :])
                    rec = smp.tile([128, 1], F32, name=f"rec_{s}_{pc}", tag="rec")
                    nc.vector.reciprocal(rec[:, :], ssum[:, :])
                    wt_ = pp_.tile([128, P], F32, name=f"wgt_{s}_{pc}", tag=f"wgt_{s}_{pc}")
                    nc.vector.tensor_scalar_mul(wt_[:, :], ex[:, :], rec[:, :])
                    wgt[(s, pc)] = wt_

            # ---- g projection + gating + residual, d-pair slabs ----
            with tc.tile_pool(name="psC", bufs=1, space="PSUM") as psC:
                for dp in range(DP):
                    dcol = dp * 1024
                    gps = {}
                    for d01 in range(2):
                        for (s, pc) in GRP:
                            gps[(d01, s, pc)] = psC.tile(
                                [128, 512], F32, name=f"ps_g_{dp}_{d01}_{s}_{pc}",
                                tag="g", bufs=8)
                    for k in range(KT):
                        gt = gp.tile([128, 1024], BF16, name=f"gt_{dp}_{k}", tag="gt")
                        nc.sync.dma_start(out=gt[:, :],
                                          in_=gw_d[k * 128:(k + 1) * 128, dcol:dcol + 1024])
                        for (s, pc) in GRP:
                            col = s * P + pc * 128
                            for d01 in range(2):
                                nc.tensor.matmul(gps[(d01, s, pc)][:, :],
                                                 ptbf[k][:, col:col + 128],
                                                 gt[:, d01 * 512:(d01 + 1) * 512],
                                                 start=(k == 0), stop=(k == KT - 1))
                    for (s, pc) in GRP:
                        row = s * P + pc * 128
                        en = ep.tile([128, 1024], F32, name=f"en_{dp}_{s}_{pc}", tag="en")
                        for d01 in range(2):
                            g_ps = gps[(d01, s, pc)]
                            if gb_sb is not None:
                                nc.vector.tensor_add(
                                    g_ps[:, :], g_ps[:, :],
                                    gb_sb[0:1, dcol + d01 * 512:dcol + (d01 + 1) * 512]
                                    .partition_broadcast(128))
                            for q in range(2):
                                lo = d01 * 512 + q * 256
                                nc.vector.tensor_mul(en[:, lo:lo + 256],
                                                     g_ps[:, q * 256:(q + 1) * 256],
                                                     wgt[(s, pc)][:, :])
                        pn = pnp.tile([128, 1024], F32, name=f"pn_{dp}_{s}_{pc}", tag="pn")
                        nc.scalar.dma_start(out=pn[:, :],
                                            in_=pnat_d[row:row + 128, dcol:dcol + 1024])
                        nc.vector.tensor_add(en[:, :], en[:, :], pn[:, :])
                        nc.scalar.dma_start(out=out_d[row:row + 128, dcol:dcol + 1024],
                                            in_=en[:, :])

    nc.compile()
    _built[key] = nc
    return nc


def kernel(**inputs):
    global LAST_RESULTS
    x = np.ascontiguousarray(inputs["x"], dtype=np.float32)
    tw = np.ascontiguousarray(inputs["theta_w"], dtype=np.float32)
    fw = np.ascontiguousarray(inputs["f_w"], dtype=np.float32)
    gw = np.ascontiguousarray(inputs["g_w"], dtype=np.float32)
    tb = np.asarray(inputs["theta_b"], dtype=np.float32)
    fb = np.asarray(inputs["f_b"], dtype=np.float32)
    gb = np.asarray(inputs["g_b"], dtype=np.float32)
    scale = float(np.asarray(inputs["scale"], dtype=np.float32).reshape(-1)[0])

    with_tb = bool(np.any(tb))
    with_fb = bool(np.any(fb))
    with_gb = bool(np.any(gb))
    nc = _build(with_tb, with_fb, with_gb)

    # patchify: [B,C,H,W] -> [B,P,D] with D ordered (c, u, v)
    p = x.reshape(B, C, NPS, PH, NPS, PW).transpose(0, 2, 4, 1, 3, 5).reshape(B, P, D)
    gw_b = np.ascontiguousarray(scale * gw).astype(ml_dtypes.bfloat16)
    in_maps = []
    for ci in range(NCORES):
        p2 = p[ci * SPC:(ci + 1) * SPC]                      # [SPC, P, D]
        pnat = np.ascontiguousarray(p2.reshape(PP, D), dtype=np.float32)
        pT = np.ascontiguousarray(p2.transpose(2, 0, 1).reshape(D, PP), dtype=np.float32)
        m = {"pt32": pT, "ptbf": pT.astype(ml_dtypes.bfloat16), "pnat": pnat,
             "tw": tw, "fw": fw, "gw": gw_b}
        if with_tb:
            m["tb"] = np.ascontiguousarray(tb.reshape(E, 1))
        if with_fb:
            m["fb"] = np.ascontiguousarray(fb.reshape(E, 1))
        if with_gb:
            m["gb"] = np.ascontiguousarray((scale * gb).reshape(1, D))
        in_maps.append(m)

    res = run_bass_kernel_spmd(nc, in_maps, core_ids=list(range(NCORES)))
    LAST_RESULTS = res
    o = np.concatenate([res.results[ci]["out"].reshape(SPC, P, D)
                        for ci in range(NCORES)], axis=0)     # [B, P, D]
    img = (o.reshape(B, NPS, NPS, C, PH, PW)
            .transpose(0, 3, 1, 4, 2, 5)
            .reshape(B, C, H, W))
    return np.ascontiguousarray(img, dtype=np.float32)


# revision 3
# speedup vs baseline: 1.4464x; 1.1283x over previous
"""Trainium2 Bass kernel for nn_LocalEnhancementModule (8-core SPMD, data-parallel over batch).

Per-sample computation (B=16, P=256 patches, D=4096, E=512):
    p      = patchify(x)                       [P, D]
    theta  = p @ theta_w + theta_b             [P, E]
    f      = p @ f_w + f_b                     [P, E]
    wgt    = softmax(theta @ f.T, axis=-1)     [P, P]
    g      = p @ g_w + g_b                     [P, D]
    out    = unpatchify(wgt[:,None,:] * g.reshape(P,C,P)) * scale + x

Sharding: 2 samples per core. Host pre-patchifies x and uploads a dense
fp16 pT (moving operand for theta/f, stationary for g), fp16 theta/f/g
weights, and fp32 p_nat for the residual. scale is folded into g_w on the
host. fp16 keeps ~tf32-class input precision (10-bit mantissa) at half the
HBM traffic of float32r, with fp32 PSUM accumulation throughout; softmax
runs in fp32.

Schedule: theta/f run k-outer into 8 PSUM banks (weights stream as full
[128,512] row-tiles); scores+softmax; then the g projection streams g_w
column slabs with 8 PSUM accumulators rotating over single-d rounds so two
rounds are always in flight. DMA issue is split across both HWDGE rings
(sync: pt16/theta_w/f_w/g_w; scalar: p_nat/out).
"""

import sys
import numpy as np

try:
    import concourse.bacc as bacc
except ImportError:  # pragma: no cover
    for _p in ("/opt/trn_rl_repo", "/root/.axon_site/_ro/trn_rl_repo"):
        if _p not in sys.path:
            sys.path.append(_p)
    import concourse.bacc as bacc
import concourse.mybir as mybir
import concourse.tile as tile
from concourse.bass_utils import run_bass_kernel_spmd

NCORES = 8
B, C, H, W = 16, 16, 256, 256
NPS, PH, PW = 16, 16, 16
P = NPS * NPS            # 256 patches
D = C * PH * PW          # 4096
E = 512
SPC = B // NCORES        # 2 samples per core
PP = SPC * P             # 512 patch rows per core
KT = D // 128            # 32 contraction tiles
ET = E // 128            # 4 embedding chunks
DCH = D // 512           # 8 column chunks for g
GRP = [(s, pc) for s in range(SPC) for pc in range(2)]

F32 = mybir.dt.float32
F16 = mybir.dt.float16

_built = {}
LAST_RESULTS = None  # stashed BassKernelResults for test harness introspection


def _build(with_tb, with_fb, with_gb):
    key = (with_tb, with_fb, with_gb)
    if key in _built:
        return _built[key]

    nc = bacc.Bacc("TRN2", num_devices=NCORES, debug=False)
    pt16_d = nc.dram_tensor("pt16", [D, PP], F16, kind="ExternalInput").ap()
    pnat_d = nc.dram_tensor("pnat", [PP, D], F32, kind="ExternalInput").ap()
    tw_d = nc.dram_tensor("tw", [D, E], F16, kind="ExternalInput").ap()
    fw_d = nc.dram_tensor("fw", [D, E], F16, kind="ExternalInput").ap()
    gw_d = nc.dram_tensor("gw", [D, D], F16, kind="ExternalInput").ap()
    tb_d = nc.dram_tensor("tb", [E, 1], F32, kind="ExternalInput").ap() if with_tb else None
    fb_d = nc.dram_tensor("fb", [E, 1], F32, kind="ExternalInput").ap() if with_fb else None
    gb_d = nc.dram_tensor("gb", [1, D], F32, kind="ExternalInput").ap() if with_gb else None
    out_d = nc.dram_tensor("out", [PP, D], F32, kind="ExternalOutput").ap()

    with tile.TileContext(nc) as tc:
        with tc.tile_pool(name="persist", bufs=1) as pp_, \
             tc.tile_pool(name="wstream", bufs=8) as wp, \
             tc.tile_pool(name="gstream", bufs=8) as gp, \
             tc.tile_pool(name="pnstream", bufs=4) as pnp, \
             tc.tile_pool(name="enh", bufs=6) as ep, \
             tc.tile_pool(name="sm", bufs=2) as smp:

            bias_sb = {0: [], 1: []}
            for wi, bd in ((0, tb_d), (1, fb_d)):
                if bd is None:
                    continue
                for e in range(ET):
                    bt = pp_.tile([128, 1], F32, name=f"bias_{wi}_{e}", tag=f"bias_{wi}_{e}")
                    nc.scalar.dma_start(out=bt[:, :], in_=bd[e * 128:(e + 1) * 128, :])
                    bias_sb[wi].append(bt)
            gb_sb = None
            if gb_d is not None:
                gb_sb = pp_.tile([1, D], F32, name="gb_sb", tag="gb_sb")
                nc.scalar.dma_start(out=gb_sb[:, :], in_=gb_d[:, :])

            # ---- theta / f projections, k-outer into 8 PSUM banks ----
            # projT[(w,e)] = [128(e), PP]  (thetaT / fT, fp16)
            pt16 = []
            with tc.tile_pool(name="psA", bufs=1, space="PSUM") as psA:
                ps_attn = {}
                for wi in (0, 1):
                    for e in range(ET):
                        ps_attn[(wi, e)] = psA.tile([128, PP], F32,
                                                    name=f"ps_attn_{wi}_{e}",
                                                    tag=f"attn_{wi}_{e}")
                for k in range(KT):
                    t16 = pp_.tile([128, PP], F16, name=f"pt16_{k}", tag=f"pt16_{k}")
                    nc.sync.dma_start(out=t16[:, :], in_=pt16_d[k * 128:(k + 1) * 128, :])
                    pt16.append(t16)
                    for wi, wd in ((0, tw_d), (1, fw_d)):
                        wt = wp.tile([128, E], F16, name=f"wt_{wi}_{k}", tag="w")
                        nc.sync.dma_start(out=wt[:, :], in_=wd[k * 128:(k + 1) * 128, :])
                        for e in range(ET):
                            nc.tensor.matmul(ps_attn[(wi, e)][:, :],
                                             wt[:, e * 128:(e + 1) * 128],
                                             t16[:, :],
                                             start=(k == 0), stop=(k == KT - 1))
                proj_sb = {}
                for wi in (0, 1):
                    for e in range(ET):
                        sb = pp_.tile([128, PP], F16, name=f"proj_{wi}_{e}",
                                      tag=f"proj_{wi}_{e}")
                        if bias_sb[wi]:
                            nc.scalar.activation(sb[:, :], ps_attn[(wi, e)][:, :],
                                                 mybir.ActivationFunctionType.Identity,
                                                 bias=bias_sb[wi][e][:, :], scale=1.0)
                        elif e % 2 == 0:
                            nc.scalar.copy(sb[:, :], ps_attn[(wi, e)][:, :])
                        else:
                            nc.vector.tensor_copy(sb[:, :], ps_attn[(wi, e)][:, :])
                        proj_sb[(wi, e)] = sb

            # ---- scores + softmax per (sample, p-chunk) ----
            wgt = {}
            with tc.tile_pool(name="psB", bufs=1, space="PSUM") as psB:
                for (s, pc) in GRP:
                    sps = psB.tile([128, P], F32, name=f"ps_sc_{s}_{pc}", tag="sc", bufs=2)
                    col = s * P + pc * 128
                    for e in range(ET):
                        nc.tensor.matmul(sps[:, :],
                                         proj_sb[(0, e)][:, col:col + 128],
                                         proj_sb[(1, e)][:, s * P:(s + 1) * P],
                                         start=(e == 0), stop=(e == ET - 1))
                    mx = smp.tile([128, 1], F32, name=f"mx_{s}_{pc}", tag="mx")
                    nc.vector.tensor_reduce(out=mx[:, :], in_=sps[:, :],
                                            axis=mybir.AxisListType.X, op=mybir.AluOpType.max)
                    ngm = smp.tile([128, 1], F32, name=f"ngm_{s}_{pc}", tag="ngm")
                    nc.vector.tensor_scalar_mul(ngm[:, :], mx[:, :], -1.0)
                    ex = smp.tile([128, P], F32, name=f"ex_{s}_{pc}", tag="ex")
                    ssum = smp.tile([128, 1], F32, name=f"ssum_{s}_{pc}", tag="ssum")
                    nc.scalar.activation(ex[:, :], sps[:, :], mybir.ActivationFunctionType.Exp,
                                         bias=ngm[:, :], scale=1.0, accum_out=ssum[:, :])
                    rec = smp.tile([128, 1], F32, name=f"rec_{s}_{pc}", tag="rec")
                    nc.vector.reciprocal(rec[:, :], ssum[:, :])
                    wt_ = pp_.tile([128, P], F32, name=f"wgt_{s}_{pc}", tag=f"wgt_{s}_{pc}")
                    nc.vector.tensor_scalar_mul(wt_[:, :], ex[:, :], rec[:, :])
                    wgt[(s, pc)] = wt_

            # ---- g projection + gating + residual, single-d rounds, 2 in flight ----
            with tc.tile_pool(name="psC", bufs=1, space="PSUM") as psC:
                for d in range(DCH):
                    dcol = d * 512
                    gps = {}
                    for (s, pc) in GRP:
                        gps[(s, pc)] = psC.tile([128, 512], F32,
                                                name=f"ps_g_{d}_{s}_{pc}", tag="g", bufs=8)
                    for k in range(KT):
                        gt = gp.tile([128, 512], F16, name=f"gt_{d}_{k}", tag="gt")
                        nc.sync.dma_start(out=gt[:, :],
                                          in_=gw_d[k * 128:(k + 1) * 128, dcol:dcol + 512])
                        for (s, pc) in GRP:
                            col = s * P + pc * 128
                            nc.tensor.matmul(gps[(s, pc)][:, :],
                                             pt16[k][:, col:col + 128],
                                             gt[:, :],
                                             start=(k == 0), stop=(k == KT - 1))
                    for (s, pc) in GRP:
                        row = s * P + pc * 128
                        g_ps = gps[(s, pc)]
                        if gb_sb is not None:
                            nc.vector.tensor_add(
                                g_ps[:, :], g_ps[:, :],
                                gb_sb[0:1, dcol:dcol + 512].partition_broadcast(128))
                        en = ep.tile([128, 512], F32, name=f"en_{d}_{s}_{pc}", tag="en")
                        nc.vector.tensor_mul(en[:, 0:256], g_ps[:, 0:256], wgt[(s, pc)][:, :])
                        nc.vector.tensor_mul(en[:, 256:512], g_ps[:, 256:512], wgt[(s, pc)][:, :])
                        pn = pnp.tile([128, 512], F32, name=f"pn_{d}_{s}_{pc}", tag="pn")
                        nc.scalar.dma_start(out=pn[:, :],
                                            in_=pnat_d[row:row + 128, dcol:dcol + 512])
                        nc.vector.tensor_add(en[:, :], en[:, :], pn[:, :])
                        nc.scalar.dma_start(out=out_d[row:row + 128, dcol:dcol + 512],
                                            in_=en[:, :])

    nc.compile()
    _built[key] = nc
    return nc


def kernel(**inputs):
    global LAST_RESULTS
    x = np.ascontiguousarray(inputs["x"], dtype=np.float32)
    tw = np.asarray(inputs["theta_w"], dtype=np.float32)
    fw = np.asarray(inputs["f_w"], dtype=np.float32)
    gw = np.asarray(inputs["g_w"], dtype=np.float32)
    tb = np.asarray(inputs["theta_b"], dtype=np.float32)
    fb = np.asarray(inputs["f_b"], dtype=np.float32)
    gb = np.asarray(inputs["g_b"], dtype=np.float32)
    scale = float(np.asarray(inputs["scale"], dtype=np.float32).reshape(-1)[0])

    with_tb = bool(np.any(tb))
    with_fb = bool(np.any(fb))
    with_gb = bool(np.any(gb))
    nc = _build(with_tb, with_fb, with_gb)

    # patchify: [B,C,H,W] -> [B,P,D] with D ordered (c, u, v)
    p = x.reshape(B, C, NPS, PH, NPS, PW).transpose(0, 2, 4, 1, 3, 5).reshape(B, P, D)
    tw16 = np.ascontiguousarray(tw).astype(np.float16)
    fw16 = np.ascontiguousarray(fw).astype(np.float16)
    gw16 = np.ascontiguousarray(scale * gw).astype(np.float16)
    in_maps = []
    for ci in range(NCORES):
        p2 = p[ci * SPC:(ci + 1) * SPC]                      # [SPC, P, D]
        pnat = np.ascontiguousarray(p2.reshape(PP, D), dtype=np.float32)
        pT16 = np.ascontiguousarray(p2.transpose(2, 0, 1).reshape(D, PP)).astype(np.float16)
        m = {"pt16": pT16, "pnat": pnat, "tw": tw16, "fw": fw16, "gw": gw16}
        if with_tb:
            m["tb"] = np.ascontiguousarray(tb.reshape(E, 1))
        if with_fb:
            m["fb"] = np.ascontiguousarray(fb.reshape(E, 1))
        if with_gb:
            m["gb"] = np.ascontiguousarray((scale * gb).reshape(1, D))
        in_maps.append(m)

    res = run_bass_kernel_spmd(nc, in_maps, core_ids=list(range(NCORES)))
    LAST_RESULTS = res
    o = np.concatenate([res.results[ci]["out"].reshape(SPC, P, D)
                        for ci in range(NCORES)], axis=0)     # [B, P, D]
    img = (o.reshape(B, NPS, NPS, C, PH, PW)
            .transpose(0, 3, 1, 4, 2, 5)
            .reshape(B, C, H, W))
    return np.ascontiguousarray(img, dtype=np.float32)


# revision 5
# speedup vs baseline: 1.4596x; 1.0092x over previous
"""Trainium2 Bass kernel for nn_LocalEnhancementModule (8-core SPMD, data-parallel over batch).

Per-sample computation (B=16, P=256 patches, D=4096, E=512):
    p      = patchify(x)                       [P, D]
    theta  = p @ theta_w + theta_b             [P, E]
    f      = p @ f_w + f_b                     [P, E]
    wgt    = softmax(theta @ f.T, axis=-1)     [P, P]
    g      = p @ g_w + g_b                     [P, D]
    out    = unpatchify(wgt[:,None,:] * g.reshape(P,C,P)) * scale + x

Sharding: 2 samples per core. Host pre-patchifies x and uploads a dense
fp16 pT (moving operand for theta/f, stationary for g), fp16 theta/f/g
weights, and fp32 p_nat for the residual. scale is folded into g_w on the
host. fp16 keeps ~tf32-class input precision (10-bit mantissa) at half the
HBM traffic of float32r, with fp32 PSUM accumulation throughout; softmax
runs in fp32.

Schedule: theta/f run k-outer into 8 PSUM banks (weights stream as full
[128,512] row-tiles); scores+softmax; then the g projection streams g_w
column slabs with 8 PSUM accumulators rotating over single-d rounds so two
rounds are always in flight. DMA issue is split across both HWDGE rings
(sync: pt16/theta_w/f_w/g_w; scalar: p_nat/out).
"""

import sys
import numpy as np

try:
    import concourse.bacc as bacc
except ImportError:  # pragma: no cover
    for _p in ("/opt/trn_rl_repo", "/root/.axon_site/_ro/trn_rl_repo"):
        if _p not in sys.path:
            sys.path.append(_p)
    import concourse.bacc as bacc
import concourse.mybir as mybir
import concourse.tile as tile
from concourse.bass_utils import run_bass_kernel_spmd

NCORES = 8
B, C, H, W = 16, 16, 256, 256
NPS, PH, PW = 16, 16, 16
P = NPS * NPS            # 256 patches
D = C * PH * PW          # 4096
E = 512
SPC = B // NCORES        # 2 samples per core
PP = SPC * P             # 512 patch rows per core
KT = D // 128            # 32 contraction tiles
ET = E // 128            # 4 embedding chunks
DCH = D // 512           # 8 column chunks for g
GRP = [(s, pc) for s in range(SPC) for pc in range(2)]

F32 = mybir.dt.float32
F16 = mybir.dt.float16

_built = {}
LAST_RESULTS = None  # stashed BassKernelResults for test harness introspection


def _build(with_tb, with_fb, with_gb):
    key = (with_tb, with_fb, with_gb)
    if key in _built:
        return _built[key]

    nc = bacc.Bacc("TRN2", num_devices=NCORES, debug=False)
    pt16_d = nc.dram_tensor("pt16", [D, PP], F16, kind="ExternalInput").ap()
    pnat_d = nc.dram_tensor("pnat", [PP, D], F32, kind="ExternalInput").ap()
    tw_d = nc.dram_tensor("tw", [D, E], F16, kind="ExternalInput").ap()
    fw_d = nc.dram_tensor("fw", [D, E], F16, kind="ExternalInput").ap()
    gw_d = nc.dram_tensor("gw", [D, D], F16, kind="ExternalInput").ap()
    tb_d = nc.dram_tensor("tb", [E, 1], F32, kind="ExternalInput").ap() if with_tb else None
    fb_d = nc.dram_tensor("fb", [E, 1], F32, kind="ExternalInput").ap() if with_fb else None
    gb_d = nc.dram_tensor("gb", [1, D], F32, kind="ExternalInput").ap() if with_gb else None
    out_d = nc.dram_tensor("out", [PP, D], F32, kind="ExternalOutput").ap()

    with tile.TileContext(nc) as tc:
        with tc.tile_pool(name="persist", bufs=1) as pp_, \
             tc.tile_pool(name="wstream", bufs=8) as wp, \
             tc.tile_pool(name="gstream", bufs=8) as gp, \
             tc.tile_pool(name="pnstream", bufs=4) as pnp, \
             tc.tile_pool(name="enh", bufs=6) as ep, \
             tc.tile_pool(name="sm", bufs=2) as smp:

            bias_sb = {0: [], 1: []}
            for wi, bd in ((0, tb_d), (1, fb_d)):
                if bd is None:
                    continue
                for e in range(ET):
                    bt = pp_.tile([128, 1], F32, name=f"bias_{wi}_{e}", tag=f"bias_{wi}_{e}")
                    nc.scalar.dma_start(out=bt[:, :], in_=bd[e * 128:(e + 1) * 128, :])
                    bias_sb[wi].append(bt)
            gb_sb = None
            if gb_d is not None:
                gb_sb = pp_.tile([1, D], F32, name="gb_sb", tag="gb_sb")
                nc.scalar.dma_start(out=gb_sb[:, :], in_=gb_d[:, :])

            # ---- theta / f projections, k-outer into 8 PSUM banks ----
            # projT[(w,e)] = [128(e), PP]  (thetaT / fT, fp16)
            pt16 = []
            with tc.tile_pool(name="psA", bufs=1, space="PSUM") as psA:
                ps_attn = {}
                for wi in (0, 1):
                    for e in range(ET):
                        ps_attn[(wi, e)] = psA.tile([128, PP], F32,
                                                    name=f"ps_attn_{wi}_{e}",
                                                    tag=f"attn_{wi}_{e}")
                for k in range(KT):
                    t16 = pp_.tile([128, PP], F16, name=f"pt16_{k}", tag=f"pt16_{k}")
                    nc.scalar.dma_start(out=t16[:, :], in_=pt16_d[k * 128:(k + 1) * 128, :])
                    pt16.append(t16)
                    for wi, wd in ((0, tw_d), (1, fw_d)):
                        wt = wp.tile([128, E], F16, name=f"wt_{wi}_{k}", tag="w")
                        nc.sync.dma_start(out=wt[:, :], in_=wd[k * 128:(k + 1) * 128, :])
                        for e in range(ET):
                            nc.tensor.matmul(ps_attn[(wi, e)][:, :],
                                             wt[:, e * 128:(e + 1) * 128],
                                             t16[:, :],
                                             start=(k == 0), stop=(k == KT - 1))
                proj_sb = {}
                for wi in (0, 1):
                    for e in range(ET):
                        sb = pp_.tile([128, PP], F16, name=f"proj_{wi}_{e}",
                                      tag=f"proj_{wi}_{e}")
                        if bias_sb[wi]:
                            nc.scalar.activation(sb[:, :], ps_attn[(wi, e)][:, :],
                                                 mybir.ActivationFunctionType.Identity,
                                                 bias=bias_sb[wi][e][:, :], scale=1.0)
                        elif e % 2 == 0:
                            nc.scalar.copy(sb[:, :], ps_attn[(wi, e)][:, :])
                        else:
                            nc.vector.tensor_copy(sb[:, :], ps_attn[(wi, e)][:, :])
                        proj_sb[(wi, e)] = sb

            # ---- scores + softmax per (sample, p-chunk) ----
            wgt = {}
            with tc.tile_pool(name="psB", bufs=1, space="PSUM") as psB:
                for (s, pc) in GRP:
                    sps = psB.tile([128, P], F32, name=f"ps_sc_{s}_{pc}", tag="sc", bufs=2)
                    col = s * P + pc * 128
                    for e in range(ET):
                        nc.tensor.matmul(sps[:, :],
                                         proj_sb[(0, e)][:, col:col + 128],
                                         proj_sb[(1, e)][:, s * P:(s + 1) * P],
                                         start=(e == 0), stop=(e == ET - 1))
                    mx = smp.tile([128, 1], F32, name=f"mx_{s}_{pc}", tag="mx")
                    nc.vector.tensor_reduce(out=mx[:, :], in_=sps[:, :],
                                            axis=mybir.AxisListType.X, op=mybir.AluOpType.max)
                    ngm = smp.tile([128, 1], F32, name=f"ngm_{s}_{pc}", tag="ngm")
                    nc.vector.tensor_scalar_mul(ngm[:, :], mx[:, :], -1.0)
                    ex = smp.tile([128, P], F32, name=f"ex_{s}_{pc}", tag="ex")
                    ssum = smp.tile([128, 1], F32, name=f"ssum_{s}_{pc}", tag="ssum")
                    nc.scalar.activation(ex[:, :], sps[:, :], mybir.ActivationFunctionType.Exp,
                                         bias=ngm[:, :], scale=1.0, accum_out=ssum[:, :])
                    rec = smp.tile([128, 1], F32, name=f"rec_{s}_{pc}", tag="rec")
                    nc.vector.reciprocal(rec[:, :], ssum[:, :])
                    wt_ = pp_.tile([128, P], F32, name=f"wgt_{s}_{pc}", tag=f"wgt_{s}_{pc}")
                    nc.vector.tensor_scalar_mul(wt_[:, :], ex[:, :], rec[:, :])
                    wgt[(s, pc)] = wt_

            # ---- g projection + gating + residual, single-d rounds, 2 in flight ----
            # Last round (d = DCH-1) uses gt tiles prefetched on the scalar ring
            # into a resident set during round DCH-3, and runs k-inner per group
            # so the final gating overlaps the remaining matmuls instead of
            # draining after the PE finishes.
            LAST = DCH - 1
            gs_last = []
            with tc.tile_pool(name="psC", bufs=1, space="PSUM") as psC:

                def gate_group(d, dcol, s, pc, g_ps):
                    row = s * P + pc * 128
                    if gb_sb is not None:
                        nc.vector.tensor_add(
                            g_ps[:, :], g_ps[:, :],
                            gb_sb[0:1, dcol:dcol + 512].partition_broadcast(128))
                    en = ep.tile([128, 512], F32, name=f"en_{d}_{s}_{pc}", tag="en")
                    nc.vector.tensor_mul(en[:, 0:256], g_ps[:, 0:256], wgt[(s, pc)][:, :])
                    nc.vector.tensor_mul(en[:, 256:512], g_ps[:, 256:512], wgt[(s, pc)][:, :])
                    pn = pnp.tile([128, 512], F32, name=f"pn_{d}_{s}_{pc}", tag="pn")
                    nc.scalar.dma_start(out=pn[:, :],
                                        in_=pnat_d[row:row + 128, dcol:dcol + 512])
                    nc.vector.tensor_add(en[:, :], en[:, :], pn[:, :])
                    nc.scalar.dma_start(out=out_d[row:row + 128, dcol:dcol + 512],
                                        in_=en[:, :])

                for d in range(LAST):
                    dcol = d * 512
                    gps = {}
                    for (s, pc) in GRP:
                        gps[(s, pc)] = psC.tile([128, 512], F32,
                                                name=f"ps_g_{d}_{s}_{pc}", tag="g", bufs=8)
                    for k in range(KT):
                        gt = gp.tile([128, 512], F16, name=f"gt_{d}_{k}", tag="gt")
                        nc.sync.dma_start(out=gt[:, :],
                                          in_=gw_d[k * 128:(k + 1) * 128, dcol:dcol + 512])
                        for (s, pc) in GRP:
                            col = s * P + pc * 128
                            nc.tensor.matmul(gps[(s, pc)][:, :],
                                             pt16[k][:, col:col + 128],
                                             gt[:, :],
                                             start=(k == 0), stop=(k == KT - 1))
                    for (s, pc) in GRP:
                        gate_group(d, dcol, s, pc, gps[(s, pc)])
                    if d == DCH - 3:
                        # prefetch the last round's g_w slab on the scalar ring
                        for k in range(KT):
                            gl = pp_.tile([128, 512], F16, name=f"gs_last_{k}",
                                          tag=f"gs_last_{k}")
                            nc.scalar.dma_start(
                                out=gl[:, :],
                                in_=gw_d[k * 128:(k + 1) * 128, LAST * 512:(LAST + 1) * 512])
                            gs_last.append(gl)

                dcol = LAST * 512
                for (s, pc) in GRP:
                    col = s * P + pc * 128
                    g_ps = psC.tile([128, 512], F32,
                                    name=f"ps_g_{LAST}_{s}_{pc}", tag="g", bufs=8)
                    for k in range(KT):
                        nc.tensor.matmul(g_ps[:, :], pt16[k][:, col:col + 128],
                                         gs_last[k][:, :],
                                         start=(k == 0), stop=(k == KT - 1))
                    gate_group(LAST, dcol, s, pc, g_ps)

    nc.compile()
    _built[key] = nc
    return nc


def kernel(**inputs):
    global LAST_RESULTS
    x = np.ascontiguousarray(inputs["x"], dtype=np.float32)
    tw = np.asarray(inputs["theta_w"], dtype=np.float32)
    fw = np.asarray(inputs["f_w"], dtype=np.float32)
    gw = np.asarray(inputs["g_w"], dtype=np.float32)
    tb = np.asarray(inputs["theta_b"], dtype=np.float32)
    fb = np.asarray(inputs["f_b"], dtype=np.float32)
    gb = np.asarray(inputs["g_b"], dtype=np.float32)
    scale = float(np.asarray(inputs["scale"], dtype=np.float32).reshape(-1)[0])

    with_tb = bool(np.any(tb))
    with_fb = bool(np.any(fb))
    with_gb = bool(np.any(gb))
    nc = _build(with_tb, with_fb, with_gb)

    # patchify: [B,C,H,W] -> [B,P,D] with D ordered (c, u, v)
    p = x.reshape(B, C, NPS, PH, NPS, PW).transpose(0, 2, 4, 1, 3, 5).reshape(B, P, D)
    tw16 = np.ascontiguousarray(tw).astype(np.float16)
    fw16 = np.ascontiguousarray(fw).astype(np.float16)
    gw16 = np.ascontiguousarray(scale * gw).astype(np.float16)
    in_maps = []
    for ci in range(NCORES):
        p2 = p[ci * SPC:(ci + 1) * SPC]                      # [SPC, P, D]
        pnat = np.ascontiguousarray(p2.reshape(PP, D), dtype=np.float32)
        pT16 = np.ascontiguousarray(p2.transpose(2, 0, 1).reshape(D, PP)).astype(np.float16)
        m = {"pt16": pT16, "pnat": pnat, "tw": tw16, "fw": fw16, "gw": gw16}
        if with_tb:
            m["tb"] = np.ascontiguousarray(tb.reshape(E, 1))
        if with_fb:
            m["fb"] = np.ascontiguousarray(fb.reshape(E, 1))
        if with_gb:
            m["gb"] = np.ascontiguousarray((scale * gb).reshape(1, D))
        in_maps.append(m)

    res = run_bass_kernel_spmd(nc, in_maps, core_ids=list(range(NCORES)))
    LAST_RESULTS = res
    o = np.concatenate([res.results[ci]["out"].reshape(SPC, P, D)
                        for ci in range(NCORES)], axis=0)     # [B, P, D]
    img = (o.reshape(B, NPS, NPS, C, PH, PW)
            .transpose(0, 3, 1, 4, 2, 5)
            .reshape(B, C, H, W))
    return np.ascontiguousarray(img, dtype=np.float32)
